# revision 1
# baseline (speedup 1.0000x reference)
"""9x9 morphological dilation (sliding-window max, SAME padding) on Trainium2.

Input : label (16, 1024, 1024, 1) float32, values in [0, 1).
Output: same shape; out[b,i,j] = max over the 9x9 window centered at (i,j),
        clipped to the image (cv2-style border handling for dilate).

Strategy (per NeuronCore; batch is data-parallel over 8 cores, 2 images/core):
  - SBUF tile layout: 128 partitions x (16 rows x U cols).  Partition p holds
    img = p//64, row-block q = p%64 (image rows 16q..16q+15).  The free dim is
    r-major; each column chunk has U = cw+12 padded columns (+-4 halo, zero
    pads at image edges; zero is a valid -inf substitute since inputs >= 0).
    Chunk widths are [64, 224, 224, 224, 224, 64]: the narrow first chunk
    shortens the initial load the pipeline waits on, the narrow last chunk
    shortens the exposed final horizontal stage + stores.
  - Vertical 9-max: log tree (shifts +1,+2,+4,+1 rows) as free-dim-shifted
    tensor_max ops; the 16-row block boundaries are fed by small SBUF->SBUF
    partition-shifted DMA "carry" tiles (DVE cannot read shifted partitions).
  - Horizontal 9-max: van Herk / Gil-Werman with two masked
    tensor_tensor_scan ops (segmented running max, segment length 9; the mask
    multiplies the running state by 0 at block starts) + one merge tensor_max.
  - The vertical result R9[r] covers rows R..R+8, i.e. output row R+4; the
    recentering happens in the store DMA offsets.  Output rows 0..3 (clipped
    top windows) are built from tree intermediates and stashed into the
    otherwise-unused rows (p%64==63, r>=12) so they ride the same horizontal
    pass and stores.
  - Emission is software-pipelined: chunk i's horizontal stage is emitted
    after chunk i+1's vertical tree so the stash/carry DMA latencies hide
    under tree compute.  Loads are prefetched one chunk ahead on the ACT
    HWDGE ring; carries/stash/stores ride the SP ring.
"""

import numpy as np

B, H, W = 16, 1024, 1024
NCORES = 8
IMGS = 2            # images per core
RB = 16             # rows per partition
CHUNKS = [256, 256, 256, 256]   # output cols per chunk (sum = 1024)
assert sum(CHUNKS) == W
WIDTHS = sorted(set(CHUNKS))
UMAX = max(CHUNKS) + 12

_CACHE = {}


def _build(reps=1):
    import concourse.bacc as bacc
    import concourse.tile as tile
    import concourse.mybir as mybir

    f32 = mybir.dt.float32
    mx = mybir.AluOpType.max
    ml = mybir.AluOpType.mult

    nc = bacc.Bacc("TRN2", target_bir_lowering=False, debug=False, num_devices=1)
    x = nc.dram_tensor("x", [IMGS, H, W], f32, kind="ExternalInput").ap()
    y = nc.dram_tensor("y", [IMGS, H, W], f32, kind="ExternalOutput").ap()

    xv = [x[i].rearrange("(q r) c -> q r c", r=RB) for i in range(IMGS)]

    chunk_off = np.cumsum([0] + CHUNKS[:-1]).tolist()

    with tile.TileContext(nc) as tc:
        with (
            tc.tile_pool(name="px", bufs=2) as px,
            tc.tile_pool(name="pa", bufs=2) as pa,
            tc.tile_pool(name="pb", bufs=3) as pb,
            tc.tile_pool(name="pd", bufs=1) as pd,
            tc.tile_pool(name="ptop", bufs=1) as ptop,
            tc.tile_pool(name="pconst", bufs=1) as pconst,
        ):
            # --- persistent carry tiles (2 ping-pong sets, sized for UMAX) ---
            # rows 63 and 127 stay zero (image-bottom clamp)
            carr = []
            for s in range(2):
                cset = []
                for nm, k in (("xc1", 1), ("t2c", 2), ("t4c", 4), ("t8c", 1)):
                    t = pconst.tile([128, k * UMAX], f32, tag=f"{nm}_{s}")
                    nc.gpsimd.memset(t[:], 0.0)
                    cset.append(t.rearrange("p (j u) -> p j u", u=UMAX))
                carr.append(cset)

            # --- per-width masks for the segmented horizontal scans ---
            # Mp: 0.0 where u % U == 4 + 9k (prefix block starts)
            # Ms: 0.0 where u % U == 3 + 9k (suffix block ends)
            masks = {}
            for cw in WIDTHS:
                u = cw + 12
                mp = pconst.tile([128, RB * u], f32, tag=f"mp{cw}")
                ms = pconst.tile([128, RB * u], f32, tag=f"ms{cw}")
                mp3 = mp.rearrange("p (r u) -> p r u", u=u)
                ms3 = ms.rearrange("p (r u) -> p r u", u=u)
                nc.gpsimd.memset(mp[:], 1.0)
                nc.gpsimd.memset(ms[:], 1.0)
                nc.gpsimd.memset(mp3[:, :, 4:u:9], 0.0)
                nc.gpsimd.memset(ms3[:, :, 3:u:9], 0.0)
                masks[cw] = (mp, ms)

            xpend = {}

            def alloc_load(ch):
                cw = CHUNKS[ch]
                u = cw + 12
                c0 = chunk_off[ch]
                clo = max(0, c0 - 4)
                chi = min(W, c0 + cw + 8)
                ncols = chi - clo
                ulo = clo - (c0 - 4)
                X = px.tile([128, RB * u], f32, tag="x")
                x3 = X.rearrange("p (r u) -> p r u", u=u)
                if ulo > 0:
                    nc.vector.memset(x3[:, :, 0:ulo], 0.0)
                if ulo + ncols < u:
                    nc.vector.memset(x3[:, :, ulo + ncols:u], 0.0)
                row_groups = [(0, 4), (4, 8), (8, 12), (12, RB)] if ch == 0 else [(0, RB)]
                for rlo, rhi in row_groups:
                    for img in range(IMGS):
                        b = 64 * img
                        nc.scalar.dma_start(
                            out=x3[b:b + 64, rlo:rhi, ulo:ulo + ncols],
                            in_=xv[img][:, rlo:rhi, clo:chi],
                        )
                return x3

            def emit_tree(it):
                ch = it % len(CHUNKS)
                cw = CHUNKS[ch]
                u = cw + 12
                fs = RB * u
                x3 = xpend.pop(it)
                xc1_3, t2c_3, t4c_3, t8c_3 = carr[it % 2]

                def carry_copy(dst3, src3, nrows):
                    # dst[p] = src[p+1, 0:nrows] for p in 0..62 and 64..126
                    nc.sync.dma_start(out=dst3[0:63, 0:nrows, 0:u], in_=src3[1:64, 0:nrows, :])
                    nc.sync.dma_start(out=dst3[64:127, 0:nrows, 0:u], in_=src3[65:128, 0:nrows, :])

                T2 = pa.tile([128, fs], f32, tag="a")
                t2_3 = T2.rearrange("p (r u) -> p r u", u=u)
                if it == 0:
                    # first chunk: start on the first loaded row-quarter while
                    # the rest of the very first load is still in flight
                    nc.vector.tensor_max(t2_3[:, 0:3, :], x3[:, 0:3, :], x3[:, 1:4, :])
                    carry_copy(xc1_3, x3, 1)
                    nc.vector.tensor_max(t2_3[:, 3:7, :], x3[:, 3:7, :], x3[:, 4:8, :])
                    nc.vector.tensor_max(t2_3[:, 7:11, :], x3[:, 7:11, :], x3[:, 8:12, :])
                    nc.vector.tensor_max(t2_3[:, 11:15, :], x3[:, 11:15, :], x3[:, 12:16, :])
                else:
                    carry_copy(xc1_3, x3, 1)
                    nc.vector.tensor_max(t2_3[:, 0:15, :], x3[:, 0:15, :], x3[:, 1:16, :])
                nc.vector.tensor_max(t2_3[:, 15:16, :], x3[:, 15:16, :], xc1_3[:, 0:1, 0:u])

                T4 = pb.tile([128, fs], f32, tag="b")
                t4_3 = T4.rearrange("p (r u) -> p r u", u=u)
                carry_copy(t2c_3, t2_3, 2)
                nc.vector.tensor_max(t4_3[:, 0:14, :], t2_3[:, 0:14, :], t2_3[:, 2:16, :])
                nc.vector.tensor_max(t4_3[:, 14:16, :], t2_3[:, 14:16, :], t2c_3[:, 0:2, 0:u])

                # top rows 0..3 (vertical prefixes over rows 0..4+k) depend only
                # on X/T2/T4 -> computed early so the stash DMA hides in the tree
                TOP = ptop.tile([128, 4 * u], f32, tag="top")
                top3 = TOP.rearrange("p (r u) -> p r u", u=u)
                for img in range(IMGS):
                    b = 64 * img
                    nc.vector.tensor_max(top3[b:b + 1, 0:1, :], t4_3[b:b + 1, 0:1, :], x3[b:b + 1, 4:5, :])
                    nc.vector.tensor_max(top3[b:b + 1, 1:2, :], t4_3[b:b + 1, 0:1, :], t2_3[b:b + 1, 4:5, :])
                    nc.vector.tensor_max(top3[b:b + 1, 2:3, :], t4_3[b:b + 1, 0:1, :], t4_3[b:b + 1, 3:4, :])

                T8 = pa.tile([128, fs], f32, tag="a")
                t8_3 = T8.rearrange("p (r u) -> p r u", u=u)
                carry_copy(t4c_3, t4_3, 4)
                nc.vector.tensor_max(t8_3[:, 0:12, :], t4_3[:, 0:12, :], t4_3[:, 4:16, :])
                nc.vector.tensor_max(t8_3[:, 12:16, :], t4_3[:, 12:16, :], t4c_3[:, 0:4, 0:u])
                for img in range(IMGS):
                    b = 64 * img
                    nc.scalar.copy(top3[b:b + 1, 3:4, :], t8_3[b:b + 1, 0:1, :])

                R9 = pb.tile([128, fs], f32, tag="b")
                r9_3 = R9.rearrange("p (r u) -> p r u", u=u)
                carry_copy(t8c_3, t8_3, 1)
                nc.vector.tensor_max(r9_3[:, 0:15, :], t8_3[:, 0:15, :], t8_3[:, 1:16, :])
                nc.vector.tensor_max(r9_3[:, 15:16, :], t8_3[:, 15:16, :], t8c_3[:, 0:1, 0:u])
                for img in range(IMGS):
                    b = 64 * img
                    nc.sync.dma_start(out=r9_3[b + 63:b + 64, 12:16, :], in_=top3[b:b + 1, 0:4, :])
                return (R9, r9_3)

            def emit_hstage(it, R9, r9_3, last=False):
                ch = it % len(CHUNKS)
                cw = CHUNKS[ch]
                u = cw + 12
                fs = RB * u
                c0 = chunk_off[ch]
                MPw, MSw = masks[cw]
                PH = pa.tile([128, fs], f32, tag="a")
                SH = pd.tile([128, fs], f32, tag="d")
                hf = fs // 2
                if last:
                    # split scans by row-halves so the final stores overlap
                    # the second half's scans (shrinks the exposed tail)
                    halves = [(0, hf), (hf, fs)]
                else:
                    halves = [(0, fs)]
                for lo, hi in halves:
                    nc.vector.tensor_tensor_scan(
                        PH[:, lo:hi], MPw[:, lo:hi], R9[:, lo:hi], 0.0, op0=ml, op1=mx
                    )
                    nc.vector.tensor_tensor_scan(
                        PH[:, lo:hi][:, ::-1] if False else SH[:, lo:hi][:, ::-1],
                        MSw[:, lo:hi][:, ::-1], R9[:, lo:hi][:, ::-1], 0.0,
                        op0=ml, op1=mx,
                    )

                OUT = pb.tile([128, fs], f32, tag="b")
                o3 = OUT.rearrange("p (r u) -> p r u", u=u)
                ph3 = PH.rearrange("p (r u) -> p r u", u=u)
                sh3 = SH.rearrange("p (r u) -> p r u", u=u)
                # merge + store per 4-row group so stores start early; the
                # last chunk's stores ride the (now idle) ACT ring so the
                # final drain doesn't wait behind the SP queue
                st = nc.sync
                if not last:
                    nc.vector.tensor_max(
                        o3[:, :, 4:4 + cw], sh3[:, :, 0:cw], ph3[:, :, 8:8 + cw]
                    )
                for g in range(4):
                    r0g, r1g = 4 * g, 4 * g + 4
                    if last:
                        nc.vector.tensor_max(
                            o3[:, r0g:r1g, 4:4 + cw],
                            sh3[:, r0g:r1g, 0:cw],
                            ph3[:, r0g:r1g, 8:8 + cw],
                        )
                    for img in range(IMGS):
                        b = 64 * img
                        ymain = y[img][4:4 + 63 * RB, c0:c0 + cw].rearrange(
                            "(q r) c -> q r c", r=RB
                        )
                        st.dma_start(
                            out=ymain[:, r0g:r1g, :], in_=o3[b:b + 63, r0g:r1g, 4:4 + cw]
                        )
                        if g < 3:
                            # bottom rows 1012..1023 live at (p%64==63, r 0..11)
                            ytail = y[img][1012 + 4 * g:1016 + 4 * g, c0:c0 + cw]
                            st.dma_start(
                                out=ytail, in_=o3[b + 63:b + 64, r0g:r1g, 4:4 + cw]
                            )
                        else:
                            # top rows 0..3 live in the stash (p%64==63, r 12..15)
                            ytop = y[img][0:4, c0:c0 + cw]
                            st.dma_start(
                                out=ytop, in_=o3[b + 63:b + 64, 12:16, 4:4 + cw]
                            )

            niter = len(CHUNKS) * reps
            xpend[0] = alloc_load(0)
            pending = None
            for it in range(niter):
                if it + 1 < niter:
                    xpend[it + 1] = alloc_load((it + 1) % len(CHUNKS))
                state = emit_tree(it)
                if pending is not None:
                    emit_hstage(*pending)
                pending = (it, *state)
            emit_hstage(*pending, last=True)

    nc.compile()
    return nc


def kernel(label):
    lab = np.ascontiguousarray(
        np.asarray(label, dtype=np.float32).reshape(B, H, W)
    )
    if "nc" not in _CACHE:
        _CACHE["nc"] = _build()
    nc = _CACHE["nc"]

    from concourse.bass_utils import run_bass_kernel_spmd

    in_maps = [{"x": lab[IMGS * c:IMGS * (c + 1)]} for c in range(NCORES)]
    res = run_bass_kernel_spmd(nc, in_maps, core_ids=list(range(NCORES)))
    out = np.concatenate([res.results[c]["y"] for c in range(NCORES)], axis=0)
    return out.reshape(B, H, W, 1)



# revision 11
# speedup vs baseline: 64.4579x; 64.4579x over previous
"""9x9 morphological dilation (sliding-window max, SAME padding) on Trainium2.

Input : label (16, 1024, 1024, 1) float32, values in [0, 1).
Output: same shape; out[b,i,j] = max over the 9x9 window centered at (i,j),
        clipped to the image (cv2-style border handling for dilate).

Strategy (per NeuronCore; batch is data-parallel over 8 cores, 2 images/core):
  - fp16 datapath: inputs are converted f32->fp16 on the ACT engine right
    after load; all max-tree passes run in fp16 (DVE gets the 2x_1p perf
    mode; rel. rounding error 2^-11 << the 2e-2 tolerance); the final merge
    writes f32 on the Pool engine.  Zero is a valid -inf substitute for
    padding since inputs are >= 0.
  - SBUF layout: 128 partitions = 2 images x 64 row-blocks of 16 rows.
    Free dim r-major: (rows x u cols), u = chunk width + 8 halo cols.
  - Vertical 9-max: each partition's fp16 tile is extended to 24 rows by one
    SBUF->SBUF DMA per image (rows 0..7 of partition p+1 -> rows 16..23 of
    p; full-width so each partition is a single 3.8KB descriptor), then a
    fully local log tree: T2/T4 (+1,+2 row shifts), T8 (+4), R9 = max(T8,
    XH[r+8]).  No per-level carry DMAs.
  - Horizontal 9-max: log tree along the free dim (+1,+2,+4 col shifts, then
    merge with R9[c+8]) -- cheaper than tensor_tensor_scan, which gets no
    2x fp16 mode.
  - Work split: DVE does vT2/vT8/hG2/hG4/hG8 (fp16 2x = 0.54 ns/elem), Pool
    does vT4/vR9 and the f32 merge (0.83 ns/elem at any dtype), ACT does the
    f32->fp16 converts + load DMA issue, SP issues halo/stash/store DMAs.
  - Chunks [128, 232, 232, 232, 200]: narrow first chunk shortens the
    startup chain (load->convert->halo->tree), narrow last shortens the
    drain.  Loads are prefetched two chunks ahead (3 X slabs) so the
    convert never waits on the serialized DMA transfer queue.
  - Output rows shift by +4 (R9[q,r] = out row 16q+4+r); rows 0..3 are
    prefix maxes computed from tree taps on partition 0/64 and stashed into
    the unused rows (q=63, r=12..15) so they ride the same horizontal pass
    and stores.  Rows 1012..1023 fall out naturally via the zeroed halo.
"""

import numpy as np

B, H, W = 16, 1024, 1024
NCORES = 8
IMGS = 2            # images per core
RB = 16             # rows per partition block
HALO = 8            # vertical halo rows (window 9 -> 8)
CHUNKS = [128, 232, 232, 232, 200]   # output cols per chunk (sum = 1024)
assert sum(CHUNKS) == W

_CACHE = {}


def _build(reps=1):
    import concourse.bacc as bacc
    import concourse.tile as tile
    import concourse.mybir as mybir

    f32 = mybir.dt.float32
    f16 = mybir.dt.float16

    nc = bacc.Bacc("TRN2", target_bir_lowering=False, debug=False, num_devices=1)
    x = nc.dram_tensor("x", [IMGS, H, W], f32, kind="ExternalInput").ap()
    y = nc.dram_tensor("y", [IMGS, H, W], f32, kind="ExternalOutput").ap()

    xv = [x[i].rearrange("(q r) c -> q r c", r=RB) for i in range(IMGS)]

    nchunk = len(CHUNKS)
    chunk_off = np.cumsum([0] + CHUNKS[:-1]).tolist()
    U = [cw + 8 for cw in CHUNKS]
    UMAX = max(U)

    with tile.TileContext(nc) as tc:
        with (
            tc.tile_pool(name="px", bufs=3) as px,
            tc.tile_pool(name="pxh", bufs=1) as pxh,
            tc.tile_pool(name="pt2", bufs=1) as pt2,
            tc.tile_pool(name="pt4", bufs=1) as pt4,
            tc.tile_pool(name="pt8", bufs=1) as pt8,
            tc.tile_pool(name="pr9", bufs=2) as pr9,
            tc.tile_pool(name="pg", bufs=2) as pg,
            tc.tile_pool(name="ptop", bufs=1) as ptop,
            tc.tile_pool(name="pd", bufs=2) as pd,
            tc.tile_pool(name="pout", bufs=1) as pout,
        ):
            # two persistent fp16 input tiles (ping-pong across chunks).
            # Fully zeroed once so the full-width halo DMA and the
            # image-bottom pad rows (p=63/127) always read initialized data.
            xh_tiles = []
            for s in range(2):
                t = pxh.tile([128, (RB + HALO) * UMAX], f16, tag=f"xh{s}")
                t3 = t.rearrange("p (r u) -> p r u", u=UMAX)
                nc.gpsimd.memset(t[:], 0.0)
                xh_tiles.append(t3)

            def emit_load(it):
                ch = it % nchunk
                cw = CHUNKS[ch]
                u = U[ch]
                c0 = chunk_off[ch]
                clo = max(0, c0 - 4)
                chi = min(W, c0 + cw + 4)
                ncols = chi - clo
                ulo = clo - (c0 - 4)
                X = px.tile([128, RB * UMAX], f32, tag="x")
                x3 = X.rearrange("p (r u) -> p r u", u=UMAX)
                # left/right image-edge pad cols (slabs are recycled, so
                # re-zero on every edge chunk)
                if ulo > 0:
                    nc.vector.memset(x3[:, :, 0:ulo], 0.0)
                if ulo + ncols < u:
                    nc.vector.memset(x3[:, :, ulo + ncols:u], 0.0)
                for img in range(IMGS):
                    b = 64 * img
                    eng = nc.scalar if img == 0 else nc.sync
                    eng.dma_start(
                        out=x3[b:b + 64, :, ulo:ulo + ncols],
                        in_=xv[img][:, :, clo:chi],
                    )
                return x3

            def emit_cvt(it, x3):
                u = U[it % nchunk]
                xh3 = xh_tiles[it % 2]
                # split so the halo DMA (reads rows 0..7) starts earlier
                nc.scalar.copy(xh3[:, 0:HALO, 0:u], x3[:, 0:HALO, 0:u])
                nc.scalar.copy(xh3[:, HALO:RB, 0:u], x3[:, HALO:RB, 0:u])
                return xh3

            def emit_halo(it, xh3):
                for img in range(IMGS):
                    b = 64 * img
                    nc.sync.dma_start(
                        out=xh3[b:b + 63, RB:RB + HALO, :],
                        in_=xh3[b + 1:b + 64, 0:HALO, :],
                    )

            def emit_vtree(it, xh3):
                ch = it % nchunk
                u = U[ch]

                T2 = pt2.tile([128, 22 * UMAX], f16, tag="t2")
                t2 = T2.rearrange("p (r u) -> p r u", u=UMAX)
                nc.vector.tensor_max(t2[:, 0:22, 0:u], xh3[:, 0:22, 0:u], xh3[:, 1:23, 0:u])

                T4 = pt4.tile([128, 20 * UMAX], f16, tag="t4")
                t4 = T4.rearrange("p (r u) -> p r u", u=UMAX)
                nc.vector.tensor_max(t4[:, 0:20, 0:u], t2[:, 0:20, 0:u], t2[:, 2:22, 0:u])

                T8 = pt8.tile([128, RB * UMAX], f16, tag="t8")
                t8 = T8.rearrange("p (r u) -> p r u", u=UMAX)
                nc.vector.tensor_max(t8[:, 0:16, 0:u], t4[:, 0:16, 0:u], t4[:, 4:20, 0:u])

                # top output rows 0..3 (clipped windows): prefix maxes over
                # rows 0..4+k, from tree taps on partitions 0 (img0) / 64 (img1).
                # Emitted after T8 so they fill DVE's wait for Pool's R9.
                TOP = ptop.tile([128, 4 * UMAX], f16, tag="top")
                top = TOP.rearrange("p (r u) -> p r u", u=UMAX)
                for img in range(IMGS):
                    b = 64 * img
                    p0 = slice(b, b + 1)
                    nc.vector.tensor_max(top[p0, 0:1, 0:u], t4[p0, 0:1, 0:u], xh3[p0, 4:5, 0:u])
                    nc.vector.tensor_max(top[p0, 1:2, 0:u], t4[p0, 0:1, 0:u], t2[p0, 4:5, 0:u])
                    nc.vector.tensor_max(top[p0, 2:3, 0:u], top[p0, 1:2, 0:u], xh3[p0, 6:7, 0:u])
                    nc.vector.tensor_max(top[p0, 3:4, 0:u], t4[p0, 0:1, 0:u], t4[p0, 4:5, 0:u])

                R9 = pr9.tile([128, RB * UMAX], f16, tag="r9")
                r9 = R9.rearrange("p (r u) -> p r u", u=UMAX)
                nc.vector.tensor_max(r9[:, 0:12, 0:u], t8[:, 0:12, 0:u], xh3[:, 8:20, 0:u])
                # rows 12..15: skip p=63/127 so the top-row stash below is the
                # only writer there
                for img in range(IMGS):
                    b = 64 * img
                    nc.vector.tensor_max(
                        r9[b:b + 63, 12:16, 0:u], t8[b:b + 63, 12:16, 0:u],
                        xh3[b:b + 63, 20:24, 0:u],
                    )
                # stash top rows into unused (q=63, r=12..15)
                for img in range(IMGS):
                    b = 64 * img
                    nc.sync.dma_start(
                        out=r9[b + 63:b + 64, 12:16, 0:u], in_=top[b:b + 1, 0:4, 0:u]
                    )
                return r9

            def emit_htree(it, r9, last=False):
                ch = it % nchunk
                cw = CHUNKS[ch]
                u = U[ch]
                sub = mybir.AluOpType.subtract
                add = mybir.AluOpType.add
                relu = mybir.ActivationFunctionType.Relu

                G2 = pg.tile([128, RB * UMAX], f16, tag="g")
                g2 = G2.rearrange("p (r u) -> p r u", u=UMAX)
                nc.vector.tensor_max(g2[:, :, 0:u - 2], r9[:, :, 0:u - 2], r9[:, :, 1:u - 1])

                G4 = pg.tile([128, RB * UMAX], f16, tag="g")
                g4 = G4.rearrange("p (r u) -> p r u", u=UMAX)
                nc.vector.tensor_max(g4[:, :, 0:u - 4], g2[:, :, 0:u - 4], g2[:, :, 2:u - 2])

                G8 = pg.tile([128, RB * UMAX], f16, tag="g")
                g8 = G8.rearrange("p (r u) -> p r u", u=UMAX)
                OUT = pout.tile([128, RB * UMAX], f32, tag="out")
                o3 = OUT.rearrange("p (r u) -> p r u", u=UMAX)
                if last:
                    nc.vector.tensor_max(g8[:, :, 0:cw], g4[:, :, 0:cw], g4[:, :, 4:cw + 4])
                    nc.vector.tensor_max(o3[:, :, 0:cw], g8[:, :, 0:cw], r9[:, :, 8:cw + 8])
                    return o3
                # max(a,b) = b + relu(a-b): subtract/add on gpsimd, relu on
                # ACT -- offloads the two tail passes from the DVE
                D8 = pd.tile([128, RB * UMAX], f16, tag="d")
                d8 = D8.rearrange("p (r u) -> p r u", u=UMAX)
                nc.gpsimd.tensor_tensor(
                    d8[:, :, 0:cw], g4[:, :, 0:cw], g4[:, :, 4:cw + 4], op=sub)
                E8 = pd.tile([128, RB * UMAX], f16, tag="d")
                e8 = E8.rearrange("p (r u) -> p r u", u=UMAX)
                nc.scalar.activation(e8[:, :, 0:cw], d8[:, :, 0:cw], relu)
                nc.gpsimd.tensor_tensor(
                    g8[:, :, 0:cw], g4[:, :, 4:cw + 4], e8[:, :, 0:cw], op=add)
                D9 = pd.tile([128, RB * UMAX], f16, tag="d")
                d9 = D9.rearrange("p (r u) -> p r u", u=UMAX)
                nc.gpsimd.tensor_tensor(
                    d9[:, :, 0:cw], g8[:, :, 0:cw], r9[:, :, 8:cw + 8], op=sub)
                E9 = pd.tile([128, RB * UMAX], f16, tag="d")
                e9 = E9.rearrange("p (r u) -> p r u", u=UMAX)
                nc.scalar.activation(e9[:, :, 0:cw], d9[:, :, 0:cw], relu)
                nc.gpsimd.tensor_tensor(
                    o3[:, :, 0:cw], r9[:, :, 8:cw + 8], e9[:, :, 0:cw], op=add)
                return o3

            def emit_stores(it, o3):
                ch = it % nchunk
                cw = CHUNKS[ch]
                c0 = chunk_off[ch]
                for img in range(IMGS):
                    b = 64 * img
                    ymain = y[img][4:4 + 63 * RB, c0:c0 + cw].rearrange(
                        "(q r) c -> q r c", r=RB
                    )
                    nc.sync.dma_start(out=ymain, in_=o3[b:b + 63, :, 0:cw])
                    nc.sync.dma_start(
                        out=y[img][1012:1024, c0:c0 + cw], in_=o3[b + 63:b + 64, 0:12, 0:cw]
                    )
                    nc.sync.dma_start(
                        out=y[img][0:4, c0:c0 + cw], in_=o3[b + 63:b + 64, 12:16, 0:cw]
                    )

            # --- software-pipelined emission (loads prefetch 2 ahead) ---
            niter = nchunk * reps
            xp = {0: emit_load(0)}
            xhp = {0: emit_cvt(0, xp.pop(0))}
            emit_halo(0, xhp[0])
            if niter > 1:
                xp[1] = emit_load(1)
            for it in range(niter):
                if it + 2 < niter:
                    xp[it + 2] = emit_load(it + 2)
                if it + 1 < niter:
                    xhp[it + 1] = emit_cvt(it + 1, xp.pop(it + 1))
                    emit_halo(it + 1, xhp[it + 1])
                r9 = emit_vtree(it, xhp.pop(it))
                o3 = emit_htree(it, r9, last=(it == niter - 1))
                emit_stores(it, o3)

    nc.compile()
    return nc


def kernel(label):
    lab = np.ascontiguousarray(
        np.asarray(label, dtype=np.float32).reshape(B, H, W)
    )
    if "nc" not in _CACHE:
        _CACHE["nc"] = _build()
    nc = _CACHE["nc"]

    from concourse.bass_utils import run_bass_kernel_spmd

    in_maps = [{"x": lab[IMGS * c:IMGS * (c + 1)]} for c in range(NCORES)]
    res = run_bass_kernel_spmd(nc, in_maps, core_ids=list(range(NCORES)))
    out = np.concatenate([res.results[c]["y"] for c in range(NCORES)], axis=0)
    return out.reshape(B, H, W, 1)
